# revision 6
# baseline (speedup 1.0000x reference)
"""Trainium2 Bass kernel for nn_DCSRM (style-recalibration + dynamic depthwise conv).

Reference computation (per sample n, channel c):
    mean/std over HxW -> z = mean*cfc0 + std*cfc1 -> g = sigmoid(z)
    srm = x * g ; gap = mean(srm) = g*mean
    hid = relu(w1 @ gap + b1) ; dyn = w2 @ hid + b2   (per-sample 3x3 filters)
    out = depthwise_conv3x3(srm, dyn)  with zero padding 1

Key identity used here: conv(x*g, w) = conv(x, w*g) -- g folds into the 9
per-channel tap weights, so srm is never materialized.

Sharding: data-parallel over batch N=16 across 8 cores (2 samples/core).
Per core: 2 samples x 2 channel-halves = 4 SBUF tiles [128ch, 99*96] held
in fp32r (tf32-class rounding, ~2.5e-4; stats errors average out).
The 3x3 conv is computed as 9 shifted-read accumulations split across
engines by output row:  PE rows 0..63 via diag-weight fp32r matmuls
accumulating in PSUM (4-row chunks), DVE rows 64..83 and GPSIMD rows
84..95 via fused scalar_tensor_tensor FMAs with column-split APs.
"""
import os
import sys
import types
import contextlib
from contextlib import ExitStack

sys.path.insert(0, '/opt/trn_rl_repo')

import numpy as np

N, C, H, W = 16, 256, 96, 96
EPS = 1e-5
NPIX = H * W                      # 9216
CORES = 8
S_PER_CORE = N // CORES           # 2 samples per core
HALVES = C // 128                 # 2 channel halves

GUARD_TOP = 2                     # zero rows above the image
GUARD_BOT = 1
ROWS_BUF = GUARD_TOP + H + GUARD_BOT          # 99
XT_LEN = ROWS_BUF * W                          # 9504
DATA_OFF = GUARD_TOP * W                       # 192

# conv output-row ownership per engine
PE_ROWS = 64            # rows [0, 64): TensorE diag matmuls, 4-row psum chunks
DVE_ROWS = (64, 86)     # VectorE
POOL_ROWS = (86, 96)    # GpSimd

PE_CHUNK = 4            # psum chunk rows (4*96=384 <= 512 fp32 bank)
# staging pieces: (row0, row1, engine)
STAGE_PIECES = [(0, 32, 'pe'), (32, 64, 'pe'),
                (DVE_ROWS[0], DVE_ROWS[1], 'dve'),
                (POOL_ROWS[0], POOL_ROWS[1], 'pool')]

LAST_EXEC_NS = None
LAST_RESULTS = None
_PROGRAM_CACHE = {}


def _install_trace_hook_shim():
    """Make run_bass_kernel_spmd(trace=True) safe under axon: register the
    NTFF hook via ctypes if antenv.axon_hooks is absent. No-op otherwise."""
    try:
        import antenv.axon_hooks  # noqa: F401
        return
    except ImportError:
        pass
    try:
        import antenv
        import ctypes
    except ImportError:
        return
    so_path = '/opt/axon/libaxon_pjrt.so'

    def _build():
        if not os.path.exists(so_path):
            return None
        lib = ctypes.CDLL(so_path)
        if not hasattr(lib, 'axon_start_nrt_profile'):
            return None
        lib.axon_start_nrt_profile.argtypes = [
            ctypes.POINTER(ctypes.c_int64), ctypes.c_size_t]
        lib.axon_start_nrt_profile.restype = ctypes.c_int64
        lib.axon_stop_nrt_profile.argtypes = [ctypes.c_char_p]
        lib.axon_stop_nrt_profile.restype = ctypes.c_int64

        @contextlib.contextmanager
        def _hook(output_dir, device_ids):
            import jax
            jax.devices()
            if device_ids:
                ids = (ctypes.c_int64 * len(device_ids))(*device_ids)
                rc = lib.axon_start_nrt_profile(ids, len(device_ids))
            else:
                rc = lib.axon_start_nrt_profile(None, 0)
            if rc != 0:
                raise RuntimeError(f'axon_start_nrt_profile rc={rc}')
            try:
                yield
            finally:
                n = lib.axon_stop_nrt_profile(str(output_dir).encode())
                print(f'ntff profile: {n} file(s) -> {output_dir}',
                      file=sys.stderr)
        return _hook

    mod = types.ModuleType('antenv.axon_hooks')
    holder = {'hook': _build()}
    mod.get_axon_ntff_profile_hook = lambda: holder['hook']
    mod.set_axon_ntff_profile_hook = lambda h: holder.update(hook=h)
    sys.modules['antenv.axon_hooks'] = mod
    antenv.axon_hooks = mod


def _tap(k):
    return k // 3 - 1, k % 3 - 1          # dy, dx


def _build_program():
    from concourse import bacc, mybir, tile

    F32 = mybir.dt.float32
    F32R = mybir.dt.float32r
    OP = mybir.AluOpType
    AF = mybir.ActivationFunctionType

    nc = bacc.Bacc('TRN2', target_bir_lowering=False, debug=False,
                   num_devices=CORES)

    x_d = nc.dram_tensor('x', [S_PER_CORE, C, H, W], F32R,
                         kind='ExternalInput').ap()
    ca_d = nc.dram_tensor('constA', [128, 183], F32,
                          kind='ExternalInput').ap()
    cb_d = nc.dram_tensor('constB', [16, 2305], F32,
                          kind='ExternalInput').ap()
    out_d = nc.dram_tensor('out', [S_PER_CORE, C, H, W], F32,
                           kind='ExternalOutput').ap()

    with tile.TileContext(nc) as tc:
        with ExitStack() as ctx:
            cpool = ctx.enter_context(tc.tile_pool(name='const', bufs=1))
            xpool = ctx.enter_context(tc.tile_pool(name='x', bufs=3))
            bnpool = ctx.enter_context(tc.tile_pool(name='bn', bufs=4))
            smpool = ctx.enter_context(tc.tile_pool(name='small', bufs=4))
            dgpool = ctx.enter_context(tc.tile_pool(name='diag', bufs=2))
            gscr = ctx.enter_context(tc.tile_pool(name='gp_scr', bufs=2))
            st_pools = {
                'pe': ctx.enter_context(tc.tile_pool(name='stage_pe', bufs=3)),
                'dve': ctx.enter_context(tc.tile_pool(name='stage_dve', bufs=2)),
                'pool': ctx.enter_context(tc.tile_pool(name='stage_gp', bufs=2)),
            }
            pspool = ctx.enter_context(
                tc.tile_pool(name='psum', bufs=4, space='PSUM'))
            pssm = ctx.enter_context(
                tc.tile_pool(name='psum_s', bufs=2, space='PSUM'))

            constA = cpool.tile([128, 183], F32)
            constB = cpool.tile([16, 2305], F32)
            nc.sync.dma_start(constA[:], ca_d[:])
            nc.sync.dma_start(constB[:], cb_d[:])
            ident = constA[:, 0:128]

            def load_tile(s, h):
                xt = xpool.tile([128, XT_LEN], F32R, tag='x')
                nc.gpsimd.memset(xt[:, 0:DATA_OFF].bitcast(F32), 0)
                nc.gpsimd.memset(xt[:, DATA_OFF + NPIX:XT_LEN].bitcast(F32), 0)
                src = x_d[s, 128 * h:128 * (h + 1)].rearrange(
                    'c a b -> c (a b)')
                nc.sync.dma_start(xt[:, DATA_OFF:DATA_OFF + NPIX], src)
                xf = xt[:].bitcast(F32)
                bn6 = bnpool.tile([128, 18, 6], F32, tag='bn6')
                for g in range(18):
                    nc.vector.bn_stats(
                        bn6[:, g, :],
                        xf[:, DATA_OFF + 512 * g:DATA_OFF + 512 * (g + 1)])
                mv = bnpool.tile([128, 2], F32, tag='mv')
                nc.vector.bn_aggr(mv[:], bn6[:])
                return xt, mv

            def sample_weights(s, mvs):
                """mvs: dict h -> [128,2] (mean, pop var). Returns per-half
                (weff F32R [128,9], negweff F32 [128,9], dg F32R [128,9*128])."""
                g_cols = {}
                gap = smpool.tile([128, 2], F32, tag='gap')
                for h in (0, 1):
                    m = mvs[h]
                    std = smpool.tile([128, 1], F32, tag='std')
                    nc.scalar.activation(std[:], m[:, 1:2], AF.Sqrt,
                                         bias=constA[:, 182:183],
                                         scale=float(NPIX) / (NPIX - 1))
                    zt = smpool.tile([128, 1], F32, tag='zt')
                    nc.vector.tensor_tensor(zt[:], std[:],
                                            constA[:, 162 + h:163 + h],
                                            OP.mult)
                    nc.vector.scalar_tensor_tensor(
                        zt[:], m[:, 0:1], constA[:, 160 + h:161 + h], zt[:],
                        OP.mult, OP.add)
                    g = smpool.tile([128, 1], F32, tag='g')
                    nc.scalar.activation(g[:], zt[:], AF.Sigmoid)
                    g_cols[h] = g
                    nc.vector.tensor_tensor(gap[:, h:h + 1], g[:], m[:, 0:1],
                                            OP.mult)
                # hid = relu(w1 @ gap + b1): two [128,16]x[128,1] matmuls
                p = pssm.tile([16, 2], F32, tag='hid')
                for h in (0, 1):
                    nc.tensor.matmul(p[:, h:h + 1],
                                     lhsT=constA[:, 128 + 16 * h:144 + 16 * h],
                                     rhs=gap[:, h:h + 1],
                                     start=True, stop=True)
                hsum = smpool.tile([16, 1], F32, tag='hsum')
                nc.vector.tensor_reduce(hsum[:], p[:], axis=mybir.AxisListType.X,
                                        op=OP.add)
                hid = smpool.tile([16, 1], F32, tag='hid_sb')
                nc.scalar.activation(hid[:], hsum[:], AF.Relu,
                                     bias=constB[:, 2304:2305], scale=1.0)
                per_half = {}
                for h in (0, 1):
                    pd = pssm.tile([128, 9], F32, tag='dyn')
                    for k in range(9):
                        nc.tensor.matmul(
                            pd[:, k:k + 1],
                            lhsT=constB[:, k * 256 + 128 * h:
                                        k * 256 + 128 * h + 128],
                            rhs=hid[:], start=True, stop=True)
                    b2g = smpool.tile([128, 9], F32, tag='b2g')
                    nc.vector.tensor_scalar_mul(
                        b2g[:], constA[:, 164 + 9 * h:173 + 9 * h],
                        g_cols[h][:])
                    weff = smpool.tile([128, 9], F32R, tag='weff')
                    nc.vector.scalar_tensor_tensor(
                        weff[:], pd[:], g_cols[h][:], b2g[:], OP.mult, OP.add)
                    weff_f = weff[:].bitcast(F32)
                    negw = smpool.tile([128, 9], F32, tag='negw')
                    nc.vector.tensor_scalar_mul(negw[:], weff_f, -1.0)
                    dg = dgpool.tile([128, 9 * 128], F32R, tag='diag')
                    for k in range(9):
                        nc.vector.tensor_scalar_mul(
                            dg[:, 128 * k:128 * (k + 1)], ident,
                            weff_f[:, k:k + 1])
                    per_half[h] = (weff, negw, dg)
                return per_half

            def conv_tile(s, h, xt, weff, negw, dg):
                xf = xt[:].bitcast(F32)
                xfr = xf.rearrange('p (r c) -> p r c', c=W)   # [128,99,96]
                weff_f = weff[:].bitcast(F32)
                out_flat = out_d[s, 128 * h:128 * (h + 1)].rearrange(
                    'c a b -> c (a b)')

                for (a, b, eng) in STAGE_PIECES:
                    nrow = b - a
                    st = st_pools[eng].tile([128, nrow * W], F32,
                                            tag=f'st_{eng}')
                    str_ = st[:].rearrange('p (r c) -> p r c', c=W)

                    if eng == 'pe':
                        for rc in range(a, b, PE_CHUNK):
                            ps = pspool.tile([128, PE_CHUNK * W], F32,
                                             tag='cps')
                            for k in range(9):
                                dy, dx = _tap(k)
                                off = DATA_OFF + (rc + dy) * W + dx
                                nc.tensor.matmul(
                                    ps[:],
                                    lhsT=dg[:, 128 * k:128 * (k + 1)],
                                    rhs=xt[:, off:off + PE_CHUNK * W],
                                    start=(k == 0), stop=(k == 8))
                            nc.scalar.copy(
                                st[:, (rc - a) * W:(rc - a + PE_CHUNK) * W],
                                ps[:])
                        # fix column-wrap contamination of the flat reads:
                        # dx=+1 taps polluted out[:, y, 95] with x[y+dy+1, 0];
                        # dx=-1 taps polluted out[:, y, 0] with x[y+dy-1, 95].
                        for dy in (-1, 0, 1):
                            kp = (dy + 1) * 3 + 2
                            nc.vector.scalar_tensor_tensor(
                                str_[:, 0:nrow, W - 1:W],
                                xfr[:, a + dy + GUARD_TOP + 1:
                                    b + dy + GUARD_TOP + 1, 0:1],
                                negw[:, kp:kp + 1],
                                str_[:, 0:nrow, W - 1:W],
                                OP.mult, OP.add)
                            km = (dy + 1) * 3
                            nc.vector.scalar_tensor_tensor(
                                str_[:, 0:nrow, 0:1],
                                xfr[:, a + dy + GUARD_TOP - 1:
                                    b + dy + GUARD_TOP - 1, W - 1:W],
                                negw[:, km:km + 1],
                                str_[:, 0:nrow, 0:1],
                                OP.mult, OP.add)
                    else:
                        e = nc.vector if eng == 'dve' else nc.gpsimd
                        scr3 = None
                        if eng == 'pool':
                            scr = gscr.tile([128, nrow * W], F32, tag='gscr')
                            scr3 = scr[:].rearrange('p (r c) -> p r c', c=W)
                        for k in (4, 0, 1, 2, 3, 5, 6, 7, 8):
                            dy, dx = _tap(k)
                            if dx == 1:
                                co0, co1 = 0, W - 1
                            elif dx == -1:
                                co0, co1 = 1, W
                            else:
                                co0, co1 = 0, W
                            o_ap = str_[:, 0:nrow, co0:co1]
                            i_ap = xfr[:, a + dy + GUARD_TOP:
                                       b + dy + GUARD_TOP,
                                       co0 + dx:co1 + dx]
                            if k == 4:
                                e.tensor_scalar_mul(o_ap, i_ap,
                                                    weff_f[:, 4:5])
                            elif eng == 'dve':
                                e.scalar_tensor_tensor(
                                    o_ap, i_ap, weff_f[:, k:k + 1], o_ap,
                                    OP.mult, OP.add)
                            else:
                                # gpsimd has no scalar_tensor_tensor opcode
                                s_ap = scr3[:, 0:nrow, co0:co1]
                                e.tensor_scalar_mul(s_ap, i_ap,
                                                    weff_f[:, k:k + 1])
                                e.tensor_tensor(o_ap, o_ap, s_ap, OP.add)
                    nc.sync.dma_start(out_flat[:, a * W:b * W], st[:])

            for s in range(S_PER_CORE):
                tiles = {}
                mvs = {}
                for h in (0, 1):
                    tiles[h], mvs[h] = load_tile(s, h)
                per_half = sample_weights(s, mvs)
                for h in (0, 1):
                    weff, negw, dg = per_half[h]
                    conv_tile(s, h, tiles[h], weff, negw, dg)

    nc.compile()
    return nc


def _host_constants(cfc, w1, b1, w2, b2):
    A = np.zeros((128, 183), np.float32)
    A[:, 0:128] = np.eye(128, dtype=np.float32)
    w1T = np.ascontiguousarray(w1.T)              # [256, 16]
    A[:, 128:144] = w1T[:128]
    A[:, 144:160] = w1T[128:]
    A[:, 160] = cfc[0:128, 0]
    A[:, 161] = cfc[128:256, 0]
    A[:, 162] = cfc[0:128, 1]
    A[:, 163] = cfc[128:256, 1]
    b2r = b2.reshape(256, 9)
    A[:, 164:173] = b2r[0:128]
    A[:, 173:182] = b2r[128:256]
    A[:, 182] = EPS
    # permute w2 rows from o=c*9+k to o'=k*256+c, then transpose
    w2p = w2.reshape(256, 9, 16).transpose(1, 0, 2).reshape(2304, 16)
    B = np.zeros((16, 2305), np.float32)
    B[:, 0:2304] = w2p.T
    B[:, 2304] = b1
    return A, B


def kernel(x, cfc, w1, b1, w2, b2):
    global LAST_EXEC_NS, LAST_RESULTS
    _install_trace_hook_shim()
    from concourse.bass_utils import run_bass_kernel_spmd

    x = np.ascontiguousarray(x, dtype=np.float32)
    A, B = _host_constants(np.asarray(cfc, np.float32),
                           np.asarray(w1, np.float32),
                           np.asarray(b1, np.float32),
                           np.asarray(w2, np.float32),
                           np.asarray(b2, np.float32))

    if 'nc' not in _PROGRAM_CACHE:
        _PROGRAM_CACHE['nc'] = _build_program()
    nc = _PROGRAM_CACHE['nc']

    in_maps = [{'x': x[S_PER_CORE * i:S_PER_CORE * (i + 1)],
                'constA': A, 'constB': B} for i in range(CORES)]
    res = run_bass_kernel_spmd(nc, in_maps, list(range(CORES)))
    LAST_EXEC_NS = res.exec_time_ns
    LAST_RESULTS = res
    out = np.concatenate([res.results[i]['out'] for i in range(CORES)],
                         axis=0)
    return out.astype(np.float32, copy=False)


# revision 7
# speedup vs baseline: 3.4329x; 3.4329x over previous
"""Trainium2 Bass kernel for nn_DCSRM (style-recalibration + dynamic depthwise conv).

Reference computation (per sample n, channel c):
    mean/std over HxW -> z = mean*cfc0 + std*cfc1 -> g = sigmoid(z)
    srm = x * g ; gap = mean(srm) = g*mean
    hid = relu(w1 @ gap + b1) ; dyn = w2 @ hid + b2   (per-sample 3x3 filters)
    out = depthwise_conv3x3(srm, dyn)  with zero padding 1

Key identity used here: conv(x*g, w) = conv(x, w*g) -- g folds into the 9
per-channel tap weights, so srm is never materialized.

Sharding: data-parallel over batch N=16 across 8 cores (2 samples/core).
Per core: 2 samples x 2 channel-halves = 4 SBUF tiles [128ch, 99*96] held
in fp32r (tf32-class rounding, ~2.5e-4; stats errors average out).
The 3x3 conv is computed as 9 shifted-read accumulations split across
engines by output row:  PE rows 0..63 via diag-weight fp32r matmuls
accumulating in PSUM (4-row chunks), DVE rows 64..83 and GPSIMD rows
84..95 via fused scalar_tensor_tensor FMAs with column-split APs.
"""
import os
import sys
import types
import contextlib
from contextlib import ExitStack

sys.path.insert(0, '/opt/trn_rl_repo')

import numpy as np

N, C, H, W = 16, 256, 96, 96
EPS = 1e-5
NPIX = H * W                      # 9216
CORES = 8
S_PER_CORE = N // CORES           # 2 samples per core
HALVES = C // 128                 # 2 channel halves

GUARD_TOP = 2                     # zero rows above the image
GUARD_BOT = 1
ROWS_BUF = GUARD_TOP + H + GUARD_BOT          # 99
XT_LEN = ROWS_BUF * W                          # 9504
DATA_OFF = GUARD_TOP * W                       # 192

# conv output-row ownership per engine
PE_ROWS = 72            # rows [0, 72): TensorE diag matmuls, 4-row psum chunks
DVE_ROWS = (72, 96)     # VectorE; gpsimd's scalar-AP tensor_scalar is ~14ns/elem
                        # (hw-measured) so POOL does no conv work

PE_CHUNK = 4            # psum chunk rows (4*96=384 <= 512 fp32 bank)
# staging pieces: (row0, row1, engine)
STAGE_PIECES = [(0, 24, 'pe'), (24, 48, 'pe'), (48, 72, 'pe'),
                (DVE_ROWS[0], DVE_ROWS[1], 'dve')]

LAST_EXEC_NS = None
LAST_RESULTS = None
_PROGRAM_CACHE = {}


def _install_trace_hook_shim():
    """Make run_bass_kernel_spmd(trace=True) safe under axon: register the
    NTFF hook via ctypes if antenv.axon_hooks is absent. No-op otherwise."""
    try:
        import antenv.axon_hooks  # noqa: F401
        return
    except ImportError:
        pass
    try:
        import antenv
        import ctypes
    except ImportError:
        return
    so_path = '/opt/axon/libaxon_pjrt.so'

    def _build():
        if not os.path.exists(so_path):
            return None
        lib = ctypes.CDLL(so_path)
        if not hasattr(lib, 'axon_start_nrt_profile'):
            return None
        lib.axon_start_nrt_profile.argtypes = [
            ctypes.POINTER(ctypes.c_int64), ctypes.c_size_t]
        lib.axon_start_nrt_profile.restype = ctypes.c_int64
        lib.axon_stop_nrt_profile.argtypes = [ctypes.c_char_p]
        lib.axon_stop_nrt_profile.restype = ctypes.c_int64

        @contextlib.contextmanager
        def _hook(output_dir, device_ids):
            import jax
            jax.devices()
            if device_ids:
                ids = (ctypes.c_int64 * len(device_ids))(*device_ids)
                rc = lib.axon_start_nrt_profile(ids, len(device_ids))
            else:
                rc = lib.axon_start_nrt_profile(None, 0)
            if rc != 0:
                raise RuntimeError(f'axon_start_nrt_profile rc={rc}')
            try:
                yield
            finally:
                n = lib.axon_stop_nrt_profile(str(output_dir).encode())
                print(f'ntff profile: {n} file(s) -> {output_dir}',
                      file=sys.stderr)
        return _hook

    mod = types.ModuleType('antenv.axon_hooks')
    holder = {'hook': _build()}
    mod.get_axon_ntff_profile_hook = lambda: holder['hook']
    mod.set_axon_ntff_profile_hook = lambda h: holder.update(hook=h)
    sys.modules['antenv.axon_hooks'] = mod
    antenv.axon_hooks = mod


def _tap(k):
    return k // 3 - 1, k % 3 - 1          # dy, dx


def _build_program():
    from concourse import bacc, mybir, tile

    F32 = mybir.dt.float32
    F32R = mybir.dt.float32r
    OP = mybir.AluOpType
    AF = mybir.ActivationFunctionType

    nc = bacc.Bacc('TRN2', target_bir_lowering=False, debug=False,
                   num_devices=CORES)

    x_d = nc.dram_tensor('x', [S_PER_CORE, C, H, W], F32R,
                         kind='ExternalInput').ap()
    ca_d = nc.dram_tensor('constA', [128, 183], F32,
                          kind='ExternalInput').ap()
    cb_d = nc.dram_tensor('constB', [16, 2305], F32,
                          kind='ExternalInput').ap()
    out_d = nc.dram_tensor('out', [S_PER_CORE, C, H, W], F32,
                           kind='ExternalOutput').ap()

    with tile.TileContext(nc) as tc:
        with ExitStack() as ctx:
            cpool = ctx.enter_context(tc.tile_pool(name='const', bufs=1))
            xpool = ctx.enter_context(tc.tile_pool(name='x', bufs=3))
            bnpool = ctx.enter_context(tc.tile_pool(name='bn', bufs=4))
            smpool = ctx.enter_context(tc.tile_pool(name='small', bufs=4))
            dgpool = ctx.enter_context(tc.tile_pool(name='diag', bufs=2))
            st_pools = {
                'pe': ctx.enter_context(tc.tile_pool(name='stage_pe', bufs=4)),
                'dve': ctx.enter_context(tc.tile_pool(name='stage_dve', bufs=2)),
            }
            pspool = ctx.enter_context(
                tc.tile_pool(name='psum', bufs=4, space='PSUM'))
            pssm = ctx.enter_context(
                tc.tile_pool(name='psum_s', bufs=2, space='PSUM'))

            constA = cpool.tile([128, 183], F32)
            constB = cpool.tile([16, 2305], F32)
            nc.sync.dma_start(constA[:], ca_d[:])
            nc.sync.dma_start(constB[:], cb_d[:])
            ident = constA[:, 0:128]

            def load_tile(s, h):
                xt = xpool.tile([128, XT_LEN], F32R, tag='x')
                nc.gpsimd.memset(xt[:, 0:DATA_OFF].bitcast(F32), 0)
                nc.gpsimd.memset(xt[:, DATA_OFF + NPIX:XT_LEN].bitcast(F32), 0)
                src = x_d[s, 128 * h:128 * (h + 1)].rearrange(
                    'c a b -> c (a b)')
                nc.sync.dma_start(xt[:, DATA_OFF:DATA_OFF + NPIX], src)
                xf = xt[:].bitcast(F32)
                bn6 = bnpool.tile([128, 18, 6], F32, tag='bn6')
                for g in range(18):
                    nc.vector.bn_stats(
                        bn6[:, g, :],
                        xf[:, DATA_OFF + 512 * g:DATA_OFF + 512 * (g + 1)])
                mv = bnpool.tile([128, 2], F32, tag='mv')
                nc.vector.bn_aggr(mv[:], bn6[:])
                return xt, mv

            def sample_weights(s, mvs):
                """mvs: dict h -> [128,2] (mean, pop var). Returns per-half
                (weff F32R [128,9], negweff F32 [128,9], dg F32R [128,9*128])."""
                g_cols = {}
                gap = smpool.tile([128, 2], F32, tag='gap')
                for h in (0, 1):
                    m = mvs[h]
                    std = smpool.tile([128, 1], F32, tag='std')
                    nc.scalar.activation(std[:], m[:, 1:2], AF.Sqrt,
                                         bias=constA[:, 182:183],
                                         scale=float(NPIX) / (NPIX - 1))
                    zt = smpool.tile([128, 1], F32, tag='zt')
                    nc.vector.tensor_tensor(zt[:], std[:],
                                            constA[:, 162 + h:163 + h],
                                            OP.mult)
                    nc.vector.scalar_tensor_tensor(
                        zt[:], m[:, 0:1], constA[:, 160 + h:161 + h], zt[:],
                        OP.mult, OP.add)
                    g = smpool.tile([128, 1], F32, tag='g')
                    nc.scalar.activation(g[:], zt[:], AF.Sigmoid)
                    g_cols[h] = g
                    nc.vector.tensor_tensor(gap[:, h:h + 1], g[:], m[:, 0:1],
                                            OP.mult)
                # hid = relu(w1 @ gap + b1): two [128,16]x[128,1] matmuls
                p = pssm.tile([16, 2], F32, tag='hid')
                for h in (0, 1):
                    nc.tensor.matmul(p[:, h:h + 1],
                                     lhsT=constA[:, 128 + 16 * h:144 + 16 * h],
                                     rhs=gap[:, h:h + 1],
                                     start=True, stop=True)
                hsum = smpool.tile([16, 1], F32, tag='hsum')
                nc.vector.tensor_reduce(hsum[:], p[:], axis=mybir.AxisListType.X,
                                        op=OP.add)
                hid = smpool.tile([16, 1], F32, tag='hid_sb')
                nc.scalar.activation(hid[:], hsum[:], AF.Relu,
                                     bias=constB[:, 2304:2305], scale=1.0)
                per_half = {}
                for h in (0, 1):
                    pd = pssm.tile([128, 9], F32, tag='dyn')
                    for k in range(9):
                        nc.tensor.matmul(
                            pd[:, k:k + 1],
                            lhsT=constB[:, k * 256 + 128 * h:
                                        k * 256 + 128 * h + 128],
                            rhs=hid[:], start=True, stop=True)
                    b2g = smpool.tile([128, 9], F32, tag='b2g')
                    nc.vector.tensor_scalar_mul(
                        b2g[:], constA[:, 164 + 9 * h:173 + 9 * h],
                        g_cols[h][:])
                    weff = smpool.tile([128, 9], F32R, tag='weff')
                    nc.vector.scalar_tensor_tensor(
                        weff[:], pd[:], g_cols[h][:], b2g[:], OP.mult, OP.add)
                    weff_f = weff[:].bitcast(F32)
                    negw = smpool.tile([128, 9], F32, tag='negw')
                    nc.vector.tensor_scalar_mul(negw[:], weff_f, -1.0)
                    dg = dgpool.tile([128, 9 * 128], F32R, tag='diag')
                    for k in range(9):
                        nc.vector.tensor_scalar_mul(
                            dg[:, 128 * k:128 * (k + 1)], ident,
                            weff_f[:, k:k + 1])
                    per_half[h] = (weff, negw, dg)
                return per_half

            def conv_tile(s, h, xt, weff, negw, dg):
                xf = xt[:].bitcast(F32)
                xfr = xf.rearrange('p (r c) -> p r c', c=W)   # [128,99,96]
                weff_f = weff[:].bitcast(F32)
                out_flat = out_d[s, 128 * h:128 * (h + 1)].rearrange(
                    'c a b -> c (a b)')

                for (a, b, eng) in STAGE_PIECES:
                    nrow = b - a
                    st = st_pools[eng].tile([128, nrow * W], F32,
                                            tag=f'st_{eng}')
                    str_ = st[:].rearrange('p (r c) -> p r c', c=W)

                    if eng == 'pe':
                        for rc in range(a, b, PE_CHUNK):
                            ps = pspool.tile([128, PE_CHUNK * W], F32,
                                             tag='cps')
                            for k in range(9):
                                dy, dx = _tap(k)
                                off = DATA_OFF + (rc + dy) * W + dx
                                nc.tensor.matmul(
                                    ps[:],
                                    lhsT=dg[:, 128 * k:128 * (k + 1)],
                                    rhs=xt[:, off:off + PE_CHUNK * W],
                                    start=(k == 0), stop=(k == 8))
                            nc.scalar.copy(
                                st[:, (rc - a) * W:(rc - a + PE_CHUNK) * W],
                                ps[:])
                        # fix column-wrap contamination of the flat reads:
                        # dx=+1 taps polluted out[:, y, 95] with x[y+dy+1, 0];
                        # dx=-1 taps polluted out[:, y, 0] with x[y+dy-1, 95].
                        for dy in (-1, 0, 1):
                            kp = (dy + 1) * 3 + 2
                            nc.vector.scalar_tensor_tensor(
                                str_[:, 0:nrow, W - 1:W],
                                xfr[:, a + dy + GUARD_TOP + 1:
                                    b + dy + GUARD_TOP + 1, 0:1],
                                negw[:, kp:kp + 1],
                                str_[:, 0:nrow, W - 1:W],
                                OP.mult, OP.add)
                            km = (dy + 1) * 3
                            nc.vector.scalar_tensor_tensor(
                                str_[:, 0:nrow, 0:1],
                                xfr[:, a + dy + GUARD_TOP - 1:
                                    b + dy + GUARD_TOP - 1, W - 1:W],
                                negw[:, km:km + 1],
                                str_[:, 0:nrow, 0:1],
                                OP.mult, OP.add)
                    else:
                        e = nc.vector
                        for k in (4, 0, 1, 2, 3, 5, 6, 7, 8):
                            dy, dx = _tap(k)
                            if dx == 1:
                                co0, co1 = 0, W - 1
                            elif dx == -1:
                                co0, co1 = 1, W
                            else:
                                co0, co1 = 0, W
                            o_ap = str_[:, 0:nrow, co0:co1]
                            i_ap = xfr[:, a + dy + GUARD_TOP:
                                       b + dy + GUARD_TOP,
                                       co0 + dx:co1 + dx]
                            if k == 4:
                                e.tensor_scalar_mul(o_ap, i_ap,
                                                    weff_f[:, 4:5])
                            else:
                                e.scalar_tensor_tensor(
                                    o_ap, i_ap, weff_f[:, k:k + 1], o_ap,
                                    OP.mult, OP.add)
                    nc.sync.dma_start(out_flat[:, a * W:b * W], st[:])

            for s in range(S_PER_CORE):
                tiles = {}
                mvs = {}
                for h in (0, 1):
                    tiles[h], mvs[h] = load_tile(s, h)
                per_half = sample_weights(s, mvs)
                for h in (0, 1):
                    weff, negw, dg = per_half[h]
                    conv_tile(s, h, tiles[h], weff, negw, dg)

    nc.compile()
    return nc


def _host_constants(cfc, w1, b1, w2, b2):
    A = np.zeros((128, 183), np.float32)
    A[:, 0:128] = np.eye(128, dtype=np.float32)
    w1T = np.ascontiguousarray(w1.T)              # [256, 16]
    A[:, 128:144] = w1T[:128]
    A[:, 144:160] = w1T[128:]
    A[:, 160] = cfc[0:128, 0]
    A[:, 161] = cfc[128:256, 0]
    A[:, 162] = cfc[0:128, 1]
    A[:, 163] = cfc[128:256, 1]
    b2r = b2.reshape(256, 9)
    A[:, 164:173] = b2r[0:128]
    A[:, 173:182] = b2r[128:256]
    A[:, 182] = EPS
    # permute w2 rows from o=c*9+k to o'=k*256+c, then transpose
    w2p = w2.reshape(256, 9, 16).transpose(1, 0, 2).reshape(2304, 16)
    B = np.zeros((16, 2305), np.float32)
    B[:, 0:2304] = w2p.T
    B[:, 2304] = b1
    return A, B


def kernel(x, cfc, w1, b1, w2, b2):
    global LAST_EXEC_NS, LAST_RESULTS
    _install_trace_hook_shim()
    from concourse.bass_utils import run_bass_kernel_spmd

    x = np.ascontiguousarray(x, dtype=np.float32)
    A, B = _host_constants(np.asarray(cfc, np.float32),
                           np.asarray(w1, np.float32),
                           np.asarray(b1, np.float32),
                           np.asarray(w2, np.float32),
                           np.asarray(b2, np.float32))

    if 'nc' not in _PROGRAM_CACHE:
        _PROGRAM_CACHE['nc'] = _build_program()
    nc = _PROGRAM_CACHE['nc']

    in_maps = [{'x': x[S_PER_CORE * i:S_PER_CORE * (i + 1)],
                'constA': A, 'constB': B} for i in range(CORES)]
    res = run_bass_kernel_spmd(nc, in_maps, list(range(CORES)))
    LAST_EXEC_NS = res.exec_time_ns
    LAST_RESULTS = res
    out = np.concatenate([res.results[i]['out'] for i in range(CORES)],
                         axis=0)
    return out.astype(np.float32, copy=False)
